# revision 1
# baseline (speedup 1.0000x reference)
"""GroupFC kernel for Trainium2, data-parallel across 8 NeuronCores.

Problem: out = data @ W.T + b
  data: [32768, 1024] f32, W: [1024, 1024] f32 (block-diagonal-masked), b: [1024] f32

Strategy:
  - Shard batch dim across 8 cores (4096 rows each); replicate W, b.
  - Host-side: cast data shard + W to bf16, pre-transpose so the contraction
    dim (in_features) lands on SBUF partitions; broadcast b to [128, 1024].
  - On-chip per core: out_tile[128b, 512o] accumulated over 8 K-tiles in PSUM
    (PE array, bf16 operands, fp32 accumulate), bias added during the
    PSUM->SBUF evacuation on DVE, stores in natural [batch, out] layout.
"""

import os
import sys
from contextlib import ExitStack

import numpy as np

try:
    import concourse.bass as bass  # noqa: F401
except ImportError:
    sys.path.insert(0, "/opt/trn_rl_repo")

import ml_dtypes

import concourse.tile as tile
from concourse import bacc, mybir
from concourse.bass_utils import run_bass_kernel_spmd

N_CORES = 8
BATCH = 32768
SHARD = BATCH // N_CORES  # 4096
IN_DIM = 1024
OUT_DIM = 1024
P = 128
KT = IN_DIM // P  # 8 contraction tiles
NFREE = 512  # psum bank free-dim (fp32)
BB = 2048  # batch rows per load block
NBLOCKS = SHARD // BB  # 2
SUBS = BB // P  # 16 batch subtiles per block

_CACHE = {}


def _build():
    nc = bacc.Bacc("TRN2", target_bir_lowering=False, debug=False)
    dT = nc.dram_tensor(
        "dT", [IN_DIM, SHARD], mybir.dt.bfloat16, kind="ExternalInput"
    ).ap()
    wT = nc.dram_tensor(
        "wT", [IN_DIM, OUT_DIM], mybir.dt.bfloat16, kind="ExternalInput"
    ).ap()
    biasb = nc.dram_tensor(
        "biasb", [P, OUT_DIM], mybir.dt.float32, kind="ExternalInput"
    ).ap()
    out = nc.dram_tensor(
        "out", [SHARD, OUT_DIM], mybir.dt.float32, kind="ExternalOutput"
    ).ap()

    with tile.TileContext(nc) as tc:
        with ExitStack() as ctx:
            wp = ctx.enter_context(tc.tile_pool(name="w", bufs=1))
            bp = ctx.enter_context(tc.tile_pool(name="bias", bufs=1))
            dp = ctx.enter_context(tc.tile_pool(name="d", bufs=2))
            pp = ctx.enter_context(tc.tile_pool(name="psum", bufs=4, space="PSUM"))
            op = ctx.enter_context(tc.tile_pool(name="o", bufs=4))

            w_tiles = []
            for k in range(KT):
                wt = wp.tile([P, OUT_DIM], mybir.dt.bfloat16, tag=f"w{k}")
                nc.sync.dma_start(out=wt[:], in_=wT[k * P : (k + 1) * P, :])
                w_tiles.append(wt)
            bias_t = bp.tile([P, OUT_DIM], mybir.dt.float32)
            nc.sync.dma_start(out=bias_t[:], in_=biasb[:, :])

            for bb in range(NBLOCKS):
                d_tiles = []
                for k in range(KT):
                    dt_t = dp.tile([P, BB], mybir.dt.bfloat16, tag=f"d{k}")
                    nc.sync.dma_start(
                        out=dt_t[:],
                        in_=dT[k * P : (k + 1) * P, bb * BB : (bb + 1) * BB],
                    )
                    d_tiles.append(dt_t)
                for sub in range(SUBS):
                    ps0 = pp.tile([P, NFREE], mybir.dt.float32, tag="ps0")
                    ps1 = pp.tile([P, NFREE], mybir.dt.float32, tag="ps1")
                    for k in range(KT):
                        lhsT = d_tiles[k][:, sub * P : (sub + 1) * P]
                        nc.tensor.matmul(
                            ps0[:],
                            lhsT,
                            w_tiles[k][:, 0:NFREE],
                            start=(k == 0),
                            stop=(k == KT - 1),
                        )
                        nc.tensor.matmul(
                            ps1[:],
                            lhsT,
                            w_tiles[k][:, NFREE:OUT_DIM],
                            start=(k == 0),
                            stop=(k == KT - 1),
                        )
                    ot = op.tile([P, OUT_DIM], mybir.dt.float32, tag="ot")
                    nc.vector.tensor_add(ot[:, 0:NFREE], ps0[:], bias_t[:, 0:NFREE])
                    nc.vector.tensor_add(
                        ot[:, NFREE:OUT_DIM], ps1[:], bias_t[:, NFREE:OUT_DIM]
                    )
                    r0 = bb * BB + sub * P
                    nc.sync.dma_start(out=out[r0 : r0 + P, :], in_=ot[:])

    nc.compile()
    return nc


def _get_nc():
    if "nc" not in _CACHE:
        _CACHE["nc"] = _build()
    return _CACHE["nc"]


def _prep_inputs(data, W, b):
    data = np.asarray(data, dtype=np.float32)
    W = np.asarray(W, dtype=np.float32)
    b = np.asarray(b, dtype=np.float32)
    wT = np.ascontiguousarray(W.astype(ml_dtypes.bfloat16).T)  # [in, out] bf16
    bias_bc = np.ascontiguousarray(
        np.broadcast_to(b[None, :], (P, OUT_DIM))
    )  # [128, 1024] f32
    in_maps = []
    for c in range(N_CORES):
        shard = data[c * SHARD : (c + 1) * SHARD]  # [4096, 1024] f32
        dT = np.ascontiguousarray(shard.astype(ml_dtypes.bfloat16).T)  # [in, batch]
        in_maps.append({"dT": dT, "wT": wT, "biasb": bias_bc})
    return in_maps


def _run(data, W, b, trace=False, **trace_kw):
    nc = _get_nc()
    in_maps = _prep_inputs(data, W, b)
    res = run_bass_kernel_spmd(nc, in_maps, list(range(N_CORES)), trace=trace, **trace_kw)
    out = np.concatenate(
        [np.asarray(res.results[c]["out"], dtype=np.float32) for c in range(N_CORES)],
        axis=0,
    )
    return out, res


def kernel(**inputs) -> np.ndarray:
    out, _ = _run(inputs["data"], inputs["W"], inputs["b"])
    return out
